# revision 17
# baseline (speedup 1.0000x reference)
"""Trainium2 Bass kernel for nn_DataTermLayer (data-term update of optical-flow).

Key observation: the reference's bilinear warp feeds *normalized* coords in
[-1, 1] straight into a pixel-space sampler, so after clipping the gather
only ever touches I1[b, 0:3, 0:3]. The whole layer reduces to elementwise
math plus a handful of per-image scalars:

  t2x = u + 2*w ; t2y = v + 2*h          (pre-division coords)
  warped = [t2x>=511][t2y>=511] * bilinear3x3(P, t2x, t2y)
  dt    = 0.1*(I2 - warped)
  out_u = u + dt*(I1[h+1,w]-I1[h,w]) ; out_v = v + dt*(I1[h,w+1]-I1[h,w])

bf16 end-to-end (2e-2 rel tolerance; measured ~2.3e-3):
  * Host casts all inputs to bf16 and pre-scales I2 by 0.1 during the cast,
    so the device I2 tile IS dt0 = 0.1*I2; flow ships as separate U/V
    planes so every bulk tensor_tensor runs packed-bf16 at the DVE 2x
    rate; outputs are bf16 U/V planes the host interleaves/upcasts.
  * Warp runs in the shifted basis s2 = t2 - 511 (bf16-safe near the mask
    threshold) with per-image coefficients G folded on the host, using two
    fused DVE-table ops: AFFINE_MUL_REDUCE  (s2x*G11+G10)*s2y  and
    AFFINE_THEN_ADD  (s2x*G01+G00)+qty.  The x>=1 / y>=1 second-cell
    terms (EX/EY) are dropped: their contribution is O(second-difference *
    (t2-1022)/511) on <0.1% of pixels, ~1e-6 in L2.
  * The 0/1 masks are exact-by-construction outside tiny edge strips:
    cols >= wd have t2x>=512 for every pixel and rows >= 256+pd have
    t2y>=512, so only a [*, 2, wd-wz] column strip and a [pd, WF] row
    strip pay a scalar_tensor_tensor compare.
  * Row gradient: PE +-1 bidiagonal bf16 shift-matmul into PSUM + one ACT
    bf16 copy out.  Column gradient: 2x bf16 DVE subtract.
  * v-channel update runs on the otherwise-idle PE as an identity
    accumulate (psV = I@fv + I@m2) + ACT bf16 copy; u-channel add on Pool.
  * A band strip redoes rows hz..255 x cols wz.. (the only region the
    rb2/3 quadrant split misses); cols < wz there are exactly masked zero.
  * DMA queues: inputs + outputs on SP; band loads + band writebacks on
    ACT; Pool/DVE never issue DMAs.

Sharding: pure data-parallel, 4 images per core across 8 cores.
"""
import sys

sys.path.insert(0, "/opt/trn_rl_repo")

import numpy as np
import ml_dtypes

import concourse.bass as bass
import concourse.mybir as mybir
from concourse.bass_utils import run_bass_kernel_spmd
from concourse.tile import TileContext

F32 = mybir.dt.float32
BF16 = mybir.dt.bfloat16
ALU = mybir.AluOpType
ACTF = mybir.ActivationFunctionType
BF = ml_dtypes.bfloat16


def build_nc(n_imgs: int = 4, n_rb: int = 4, wz: int = 253, hz: int = 253,
             wd: int = 259, pd: int = 3, legalize: bool = True):
    """One NeuronCore program: n_imgs images of [512, 512] bf16.

    wz/hz: first col/row where the warp can be nonzero (t2 >= 511
    reachable).  wd: first col where t2x >= 512 for every pixel (x-mask
    identically 1).  pd: rows 256..256+pd-1 need the y-mask compare.
    """
    assert n_rb == 4 and 225 <= hz <= 256 and 0 < wz <= 256
    assert wz < wd <= 320 and 0 <= pd <= 16
    W = 512
    H = n_rb * 128
    NBC = 256 - hz  # band compute rows per image (rows hz..255)
    NBR = NBC + 1   # band rows loaded per image (+1 for the row-shift grad)
    WF = W - wz     # warp-math columns
    NE = wd - wz    # x-mask edge columns
    nc = bass.Bass()

    NW = n_rb * W
    NBP = max(1, NBR * n_imgs)  # band partitions
    CB = 2 + 4 * n_imgs        # first band col in CC
    # inputs/outputs ship host-permuted to the SBUF layout [p, rb*W+w]
    # (partition line = one fully-contiguous 4KB DMA chunk)
    I1 = nc.dram_tensor("I1", [n_imgs, 128, NW], BF16, kind="ExternalInput")
    I2 = nc.dram_tensor("I2", [n_imgs, 128, NW], BF16, kind="ExternalInput")
    FU = nc.dram_tensor("FU", [n_imgs, 128, NW], BF16, kind="ExternalInput")
    FV = nc.dram_tensor("FV", [n_imgs, 128, NW], BF16, kind="ExternalInput")
    NCC = 2 + 4 * n_imgs + 5
    CC = nc.dram_tensor("CC", [128, NCC], F32, kind="ExternalInput")
    # host-staged warp coordinate grids s2x|s2y (each [rbl0|rbl1]) per image
    S2 = nc.dram_tensor("S2", [n_imgs, 128, 4 * WF], BF16,
                        kind="ExternalInput")
    SM = nc.dram_tensor("SM", [128, 512], BF16, kind="ExternalInput")
    # band rows hz..hz+NBR x cols wz..: bi1|bi1r|bi2|bfu|bfv|bs2x|bs2y
    BAND = nc.dram_tensor("BAND", [NBP, 7 * WF], BF16, kind="ExternalInput")
    OU = nc.dram_tensor("OU", [n_imgs, 128, NW], BF16,
                        kind="ExternalOutput")
    OV = nc.dram_tensor("OV", [n_imgs, 128, NW], BF16,
                        kind="ExternalOutput")

    with TileContext(nc) as tc:
        with (
            tc.tile_pool(name="stat", bufs=1) as pstat,
            tc.tile_pool(name="pin", bufs=4) as pin,
            tc.tile_pool(name="ptmp", bufs=3) as ptmp,
            tc.tile_pool(name="pwarp", bufs=2) as pwarp,
            tc.tile_pool(name="pband", bufs=1) as pband,
            tc.tile_pool(name="pps", bufs=1, space="PSUM") as pps,
        ):
            cc = pstat.tile([128, NCC], F32)
            sm = pstat.tile([128, 512], BF16)
            nc.sync.dma_start(sm[:], SM[:])
            nc.sync.dma_start(cc[:], CC[:])

            def cC(j):  # [128,1] column of cc
                return cc[:, j : j + 1]

            # ---------------- input DMAs (SP queue) -------------------------
            st = [dict() for _ in range(n_imgs)]
            for b in range(n_imgs):
                s = st[b]
                for nm, src in (("i1", I1), ("i2", I2)):
                    s[nm] = pin.tile([128, NW], BF16, tag=nm, bufs=4,
                                     name=f"{nm}_{b}")
                    nc.sync.dma_start(s[nm][:], src[b])
                s["s2"] = pin.tile([128, 4 * WF], BF16, tag="s2", bufs=4,
                                   name=f"s2_{b}")
                nc.sync.dma_start(s["s2"][:], S2[b])
                for nm, src in (("fu", FU), ("fv", FV)):
                    s[nm] = pin.tile([128, NW], BF16, tag=nm, bufs=4,
                                     name=f"{nm}_{b}")
                    nc.sync.dma_start(s[nm][:], src[b])
            # band load: one host-packed DMA (ACT queue)
            if NBC > 0:
                bnd = pband.tile([NBP, 7 * WF], BF16)
                nc.scalar.dma_start(bnd[:], BAND[:])
                bi1 = bnd[:, 0:WF]
                bi1r = bnd[:, WF : 2 * WF]
                bi2 = bnd[:, 2 * WF : 3 * WF]
                bfu = bnd[:, 3 * WF : 4 * WF]
                bfv = bnd[:, 4 * WF : 5 * WF]
                bs2x = bnd[:, 5 * WF : 6 * WF]
                bs2y = bnd[:, 6 * WF : 7 * WF]

            # ---------------- band: redo rows hz..255, cols wz.. ------------
            def emit_band():
                if NBC == 0:
                    return
                P = NBP
                bqt = pband.tile([P, WF], BF16)
                nc.scalar.activation(bqt[:], bs2x, ACTF.Identity,
                                     bias=cC(CB + 2)[:P],
                                     scale=cC(CB + 3)[:P])
                nc.vector.tensor_tensor(bqt[:], bs2y, bqt[:], ALU.mult)
                bwm = pband.tile([P, WF], BF16)
                nc.scalar.activation(bwm[:], bs2x, ACTF.Identity,
                                     bias=cC(CB)[:P],
                                     scale=cC(CB + 1)[:P])
                nc.vector.tensor_tensor(bwm[:], bwm[:], bqt[:], ALU.add)
                if NE > 0:
                    nc.vector.scalar_tensor_tensor(
                        bwm[:, 0:NE], bs2x[:, 0:NE], 0.0, bwm[:, 0:NE],
                        ALU.is_ge, ALU.mult,
                    )
                nc.vector.scalar_tensor_tensor(
                    bwm[:], bs2y, 0.0, bwm[:], ALU.is_ge, ALU.mult
                )
                nc.gpsimd.tensor_tensor(bi2, bi2, bwm[:], ALU.add)
                bg1 = pband.tile([P, WF], BF16)
                nc.vector.tensor_tensor(bg1[:], bi1r, bi1, ALU.subtract)
                bg2 = pband.tile([P, WF], BF16)
                nc.vector.tensor_tensor(
                    bg2[:, 0 : WF - 1], bi1[:, 1:WF], bi1[:, 0 : WF - 1],
                    ALU.subtract
                )
                nc.gpsimd.memset(bg2[:, WF - 1 : WF], 0.0)
                nc.gpsimd.tensor_tensor(bg1[:], bi2, bg1[:], ALU.mult)
                nc.vector.tensor_tensor(bfu, bfu, bg1[:], ALU.add)
                nc.gpsimd.tensor_tensor(bg2[:], bi2, bg2[:], ALU.mult)
                nc.vector.tensor_tensor(bfv, bfv, bg2[:], ALU.add)

            # ---------------- per-image stages ------------------------------
            def emitA(b):
                s = st[b]
                i1 = s["i1"]
                ps = pps.tile([128, NW], F32, tag="ps")
                for rb in range(n_rb):
                    dst = ps[:, rb * 512 : (rb + 1) * 512]
                    rhs = i1[:, rb * 512 : (rb + 1) * 512]
                    if rb < n_rb - 1:
                        nc.tensor.matmul(dst, sm[:, 0:128], rhs,
                                         start=True, stop=False)
                        rhs2 = i1[:, (rb + 1) * 512 : (rb + 2) * 512]
                        nc.tensor.matmul(dst, sm[:, 128:256], rhs2,
                                         start=False, stop=True)
                    else:
                        nc.tensor.matmul(dst, sm[:, 256:384], rhs,
                                         start=True, stop=True)
                g1c = ptmp.tile([128, NW], BF16, tag="g1c", bufs=3,
                                name=f"g1c_{b}")
                nc.scalar.activation(g1c[:], ps[:], ACTF.Identity,
                                     bias=0.0, scale=1.0)
                g2 = ptmp.tile([128, NW], BF16, tag="g2", bufs=3,
                               name=f"g2_{b}")
                eng = nc.gpsimd if b < 2 else nc.vector
                eng.tensor_tensor(g2[:, 0 : NW - 1], i1[:, 1:NW],
                                  i1[:, 0 : NW - 1], ALU.subtract)
                g2r = g2[:].rearrange("p (r w) -> p r w", r=n_rb)
                nc.gpsimd.memset(g2r[:, :, 511:512], 0.0)
                s["g2"], s["g1c"] = g2, g1c

            def emitW(b):
                s = st[b]
                i2, s2 = s["i2"], s["s2"]
                dtv = i2[:].rearrange("p (r w) -> p r w", r=n_rb)[:, 2:4, wz:]
                s2v = s2[:].rearrange("p (c r w) -> p c r w", c=2, r=2)
                s2x = s2v[:, 0]
                s2y = s2v[:, 1]
                gb = 2 + 4 * b
                qty = pwarp.tile([128, 2, WF], BF16, tag="qty")
                nc.scalar.activation(qty[:], s2x, ACTF.Identity,
                                     bias=cC(gb + 2), scale=cC(gb + 3))
                nc.vector.tensor_tensor(qty[:], s2y, qty[:], ALU.mult)
                wm = pwarp.tile([128, 2, WF], BF16, tag="wm")
                nc.scalar.activation(wm[:], s2x, ACTF.Identity,
                                     bias=cC(gb), scale=cC(gb + 1))
                nc.vector.tensor_tensor(wm[:], wm[:], qty[:], ALU.add)
                if NE > 0:
                    nc.vector.scalar_tensor_tensor(
                        wm[:, :, 0:NE], s2x[:, :, 0:NE], 0.0,
                        wm[:, :, 0:NE], ALU.is_ge, ALU.mult,
                    )
                if pd > 0:
                    nc.vector.scalar_tensor_tensor(
                        wm[0:pd, 0, :], s2y[0:pd, 0, :], 0.0,
                        wm[0:pd, 0, :], ALU.is_ge, ALU.mult,
                    )
                nc.vector.tensor_tensor(dtv, dtv, wm[:], ALU.add)

            HWD = NW // 2

            def wbU(b):
                if NBC > 0:
                    fu = st[b]["fu"]
                    fur = fu[:].rearrange("p (r w) -> p r w", r=n_rb)
                    nc.scalar.dma_start(
                        fur[hz - 128 : hz - 128 + NBC, 1, wz:],
                        bfu[NBR * b : NBR * b + NBC, :],
                    )

            def wbV(b):
                if NBC > 0:
                    fv = st[b]["fv"]
                    fvr = fv[:].rearrange("p (r w) -> p r w", r=n_rb)
                    nc.scalar.dma_start(
                        fvr[hz - 128 : hz - 128 + NBC, 1, wz:],
                        bfv[NBR * b : NBR * b + NBC, :],
                    )

            def emitM(b):
                # multiplies: need only i1/i2/s2 (+warp), not the flow
                s = st[b]
                i2, g2, g1c = s["i2"], s["g2"], s["g1c"]
                nc.vector.tensor_tensor(g1c[:], i2[:], g1c[:], ALU.mult)
                nc.vector.tensor_tensor(g2[:], i2[:], g2[:], ALU.mult)

            def emitF(b):
                # flow adds + band writebacks + output DMAs
                s = st[b]
                fu, fv, g2, g1c = s["fu"], s["fv"], s["g2"], s["g1c"]
                if b == 3:
                    for k, sl in enumerate((slice(0, HWD), slice(HWD, NW))):
                        nc.vector.tensor_tensor(fu[:, sl], fu[:, sl],
                                                g1c[:, sl], ALU.add)
                        nc.vector.tensor_tensor(fv[:, sl], fv[:, sl],
                                                g2[:, sl], ALU.add)
                        if k == 0:
                            wbU(b)
                            wbV(b)
                        nc.sync.dma_start(OU[b, :, sl], fu[:, sl])
                        nc.sync.dma_start(OV[b, :, sl], fv[:, sl])
                    return
                nc.gpsimd.tensor_tensor(fu[:], fu[:], g1c[:], ALU.add)
                if b == 2:
                    nc.gpsimd.tensor_tensor(fv[:], fv[:], g2[:], ALU.add)
                    wbV(b)
                    s["ov"] = fv
                    wbU(b)
                    nc.sync.dma_start(OU[b], fu[:])
                    nc.sync.dma_start(OV[b], fv[:])
                    return
                wbV(b)  # band-corrected v rows before the PE reads fv
                psV = pps.tile([128, NW], F32, tag="psV")
                for rb in range(n_rb):
                    dst = psV[:, rb * 512 : (rb + 1) * 512]
                    nc.tensor.matmul(dst, sm[:, 384:512],
                                     fv[:, rb * 512 : (rb + 1) * 512],
                                     start=True, stop=False)
                    nc.tensor.matmul(dst, sm[:, 384:512],
                                     g2[:, rb * 512 : (rb + 1) * 512],
                                     start=False, stop=True)
                ov = ptmp.tile([128, NW], BF16, tag="ov", bufs=2,
                               name=f"ov_{b}")
                nc.scalar.activation(ov[:], psV[:], ACTF.Identity,
                                     bias=0.0, scale=1.0)
                wbU(b)
                nc.sync.dma_start(OU[b], fu[:])
                nc.sync.dma_start(OV[b], ov[:])

            emitA(0)
            emitW(0)
            emit_band()
            emitM(0)
            emitA(1)
            emitW(1)
            emitF(0)
            emitM(1)
            emitA(2)
            emitW(2)
            emitF(1)
            emitM(2)
            emitA(3)
            emitW(3)
            emitM(3)
            emitF(2)
            emitF(3)
    if legalize:
        legalize_single_wait(nc)
    return nc


# ---------------------------------------------------------------------------
# Post-pass: this walrus build encodes a single sync-wait slot per TPB
# instruction. Tile's sem assignment can emit 2+ waits on one instruction;
# hoist all but the last wait onto same-engine EventSemaphore carriers placed
# immediately before it (the sequencer then waits sequentially, which is
# semantically identical).
def legalize_single_wait(nc):
    import bass_rust

    capped = {
        mybir.EngineType.Activation,
        mybir.EngineType.DVE,
        mybir.EngineType.Pool,
        mybir.EngineType.PE,
        mybir.EngineType.SP,
    }
    exempt = {"EventSemaphore", "NoOp", "TriggerDma"}
    n = 0
    for fn in nc.m.functions:
        for blk in fn.blocks:
            insts = blk.instructions  # live list
            rebuilt = []
            changed = False
            for inst in list(insts):
                si = inst.sync_info
                waits = list(si.on_wait) if si is not None else []
                if (
                    len(waits) > 1
                    and inst.engine in capped
                    and str(inst.opcode) not in exempt
                ):
                    for w in waits[:-1]:
                        ev = mybir.InstEventSemaphore(
                            name=f"waitcarrier_{inst.name}_{n}", ins=[], outs=[]
                        )
                        ev.engine = inst.engine
                        ev.sync_info = bass_rust.SyncInfo(
                            on_wait=[w], on_update=[]
                        )
                        rebuilt.append(ev)
                        n += 1
                    inst.sync_info = bass_rust.SyncInfo(
                        on_wait=[waits[-1]], on_update=list(si.on_update)
                    )
                    changed = True
                rebuilt.append(inst)
            if changed:
                insts[:] = rebuilt
    return n


def _img_G(P3: np.ndarray):
    """Shifted-basis warp coefficients for one image's 3x3 corner P3[y,x].

    wm = (G01*s2x + G00) + (G11*s2x + G10)*s2y,  s2 = t2 - 511,
    wm = -0.1*warped (EX/EY second-cell terms dropped)."""
    P = P3.astype(np.float64)
    E = np.stack([P[:, 0], P[:, 1] - P[:, 0], P[:, 2] - P[:, 1]], axis=1)
    D = np.stack([E[0], E[1] - E[0], E[2] - E[1]], axis=0)
    r = 1.0 / 511.0
    Mx = np.array([[1.0, 0.0, 0.0], [-1.0, r, -r], [0.0, 0.0, r]])
    F = -0.1 * (Mx.T @ D @ Mx)
    G00 = F[0, 0] + 511.0 * (F[0, 1] + F[1, 0]) + 511.0 * 511.0 * F[1, 1]
    G01 = F[0, 1] + 511.0 * F[1, 1]
    G10 = F[1, 0] + 511.0 * F[1, 1]
    G11 = F[1, 1]
    return [np.float32(G00), np.float32(G01), np.float32(G10),
            np.float32(G11)]


def host_consts(I1c: np.ndarray, hz: int) -> np.ndarray:
    """[128, 2 + 4*n + 5] f32: cols 2+4b..5+4b: image b's G00,G01,G10,G11;
    cols CB..CB+3: band G (partition NBR*b+r holds image b's values)."""
    n_imgs = I1c.shape[0]
    CB = 2 + 4 * n_imgs
    cc = np.zeros((128, CB + 5), dtype=np.float32)
    allG = []
    for b in range(n_imgs):
        G = _img_G(I1c[b, 0:3, 0:3])
        allG.append(G)
        cc[:, 2 + 4 * b : 6 + 4 * b] = np.array(G, dtype=np.float32)[None, :]
    nbr = 257 - hz
    for b in range(n_imgs):
        for r in range(nbr):
            pp = nbr * b + r
            if pp < 128:
                cc[pp, CB : CB + 4] = allG[b]
    return cc


def host_s2(fub, fvb, wz):
    """[n, 128, 4*WF] bf16 warp grids: s2x = u + (2w-511) and
    s2y = v + (2h-511) on the quadrant rows 256.., cols wz.., each as
    [rbl0|rbl1] in the SBUF partition layout."""
    n = fub.shape[0]
    wgrid = (2.0 * np.arange(wz, 512, dtype=np.float32) - 511.0).astype(
        np.float32
    )
    hgrid = (2.0 * np.arange(512, dtype=np.float32) - 511.0).astype(
        np.float32
    )
    uq = fub.astype(np.float32)[:, 256:, wz:] + wgrid[None, None, :]
    vq = fvb.astype(np.float32)[:, 256:, wz:] + hgrid[None, 256:, None]
    WF = 512 - wz
    out = np.empty((n, 2, 2, 128, WF), dtype=np.float32)
    out[:, 0] = uq.reshape(n, 2, 128, WF)
    out[:, 1] = vq.reshape(n, 2, 128, WF)
    return np.ascontiguousarray(
        out.transpose(0, 3, 1, 2, 4).reshape(n, 128, 4 * WF)
    ).astype(BF)


def host_sm() -> np.ndarray:
    """[128, 512] bf16: cols 0:128 = shift lhsT S (S[k,m]: +1 at k=m+1,
    -1 at k=m), cols 128:256 = patch lhsT (+1 at k=0, m=127), cols
    256:384 = S with column 127 zeroed (dy row 511 must be exactly 0),
    cols 384:512 = identity (v-channel PE accumulate)."""
    sm = np.zeros((128, 512), dtype=np.float32)
    for m in range(128):
        sm[m, m] = -1.0
        if m + 1 < 128:
            sm[m + 1, m] = 1.0
    sm[0, 128 + 127] = 1.0
    sm[:, 256:384] = sm[:, 0:128]
    sm[127, 256 + 127] = 0.0
    sm[:, 384:512] = np.eye(128, dtype=np.float32)
    return sm.astype(BF)


_NC = None
_NC_KEY = None


def _get_nc(wz, hz, wd, pd):
    global _NC, _NC_KEY
    if _NC is None or _NC_KEY != (wz, hz, wd, pd):
        _NC = build_nc(4, 4, wz=wz, hz=hz, wd=wd, pd=pd)
        _NC_KEY = (wz, hz, wd, pd)
    return _NC


def _splits(flow):
    # the device sees bf16-rounded flow; all thresholds use the rounded range
    u = flow[..., 0].astype(BF).astype(np.float32)
    v = flow[..., 1].astype(BF).astype(np.float32)
    umax = float(max(u.max(), 0.0))
    vmax = float(max(v.max(), 0.0))
    umin = float(min(u.min(), 0.0))
    vmin = float(min(v.min(), 0.0))
    # first col/row where 2*x + d can reach 511.0
    wz = int(min(256, max(1, (511.0 - umax) // 2 + 1)))
    hz = int(min(256, max(225, (511.0 - vmax) // 2 + 1)))
    assert np.float32(2.0 * (wz - 1)) + np.float32(umax) < np.float32(511.0)
    assert np.float32(2.0 * (hz - 1)) + np.float32(vmax) < np.float32(511.0)
    # first col with 2*w-511+umin >= 1 (x-mask == 1 for all pixels there;
    # the +1 margin absorbs bf16 rounding of s2x)
    wd = int(np.ceil((512.0 - umin) / 2.0))
    wd = int(min(320, max(wz + 1, wd)))
    # rows 256..255+pd need the y-mask compare (2*(256+p)-511+vmin < 1)
    pd = int(max(0.0, np.ceil((-vmin) / 2.0)))
    pd = int(min(16, pd))
    return wz, hz, wd, pd


def _perm(x):
    # [n, 512, 512] row-major -> [n, 128, 2048] in the SBUF [p, rb*W+w]
    # layout (partition p holds rows p, 128+p, 256+p, 384+p)
    n = x.shape[0]
    return np.ascontiguousarray(
        x.reshape(n, 4, 128, 512).transpose(0, 2, 1, 3).reshape(n, 128, 2048)
    )


def _unperm(x):
    n = x.shape[0]
    return np.ascontiguousarray(
        x.reshape(n, 128, 4, 512).transpose(0, 2, 1, 3).reshape(n, 512, 512)
    )


def _band_pack(i1b, i2b, fub, fvb, wz, hz):
    # [NBP, 7*WF] bf16: bi1|bi1r|bi2|bfu|bfv|bs2x|bs2y
    # (rows hz..hz+NBR, cols wz..)
    n = i1b.shape[0]
    nbr = 257 - hz
    wgrid = (2.0 * np.arange(wz, 512, dtype=np.float32) - 511.0).astype(
        np.float32
    )
    hgrid = (2.0 * np.arange(512, dtype=np.float32) - 511.0).astype(
        np.float32
    )
    bs2x = (fub.astype(np.float32)[:, hz : hz + nbr, wz:]
            + wgrid[None, None, :]).astype(BF)
    bs2y = (fvb.astype(np.float32)[:, hz : hz + nbr, wz:]
            + hgrid[None, hz : hz + nbr, None]).astype(BF)
    return np.ascontiguousarray(np.concatenate(
        [
            i1b[:, hz : hz + nbr, wz:],
            i1b[:, hz + 1 : hz + 1 + nbr, wz:],
            i2b[:, hz : hz + nbr, wz:],
            fub[:, hz : hz + nbr, wz:],
            fvb[:, hz : hz + nbr, wz:],
            bs2x,
            bs2y,
        ],
        axis=2,
    ).reshape(n * nbr, -1))


def _make_in_maps(I1, I2, flow, wz, hz, n_cores=8):
    per = I1.shape[0] // n_cores
    sm = host_sm()
    i1b_all = np.asarray(I1[..., 0], dtype=np.float32).astype(BF)
    i2b_all = (np.float32(0.1) * np.asarray(I2[..., 0])).astype(BF)
    fub_all = np.asarray(flow[..., 0]).astype(BF)
    fvb_all = np.asarray(flow[..., 1]).astype(BF)
    in_maps = []
    for c in range(n_cores):
        sl = slice(c * per, (c + 1) * per)
        i1f = np.ascontiguousarray(I1[sl, :, :, 0], dtype=np.float32)
        in_maps.append(
            {
                "I1": _perm(i1b_all[sl]),
                "I2": _perm(i2b_all[sl]),
                "FU": _perm(fub_all[sl]),
                "FV": _perm(fvb_all[sl]),
                "CC": host_consts(i1f, hz),
                "S2": host_s2(fub_all[sl], fvb_all[sl], wz),
                "SM": sm,
                "BAND": _band_pack(i1b_all[sl], i2b_all[sl], fub_all[sl],
                                   fvb_all[sl], wz, hz),
            }
        )
    return in_maps


def run(I1, I2, flow, trace=False, **kw):
    wz, hz, wd, pd = _splits(np.asarray(flow))
    nc = _get_nc(wz, hz, wd, pd)
    in_maps = _make_in_maps(I1, I2, flow, wz, hz)
    res = run_bass_kernel_spmd(nc, in_maps, list(range(8)), trace=trace, **kw)
    B, H, W = I1.shape[0], I1.shape[1], I1.shape[2]
    out = np.empty((B, H, W, 2), dtype=np.float32)
    out[..., 0] = _unperm(
        np.concatenate([np.asarray(r["OU"]) for r in res.results], axis=0)
    ).astype(np.float32)
    out[..., 1] = _unperm(
        np.concatenate([np.asarray(r["OV"]) for r in res.results], axis=0)
    ).astype(np.float32)
    return out, res


def kernel(I1, I2, flow):
    out, _ = run(I1, I2, flow)
    return out.astype(np.float32)
